# revision 13
# baseline (speedup 1.0000x reference)
"""GCN 2-layer encoder on 8 trn2 NeuronCores (Bass/Tile).

Strategy (per sharding hint): partition destination nodes across 8 cores
(12500/core); each layer computes agg = A_norm @ h via per-edge gather
(dma_gather, bf16) + TensorE segment-sum using norm-scaled one-hot masks
built on VectorE, then a [128,128] GEMM (+bias matmul) and relu.
Aggregate-then-transform order: A@(xW) == (A@x)@W. Layer boundary uses an
AllGather of the bf16 h1 slices. Graph preprocessing (self loops, deg^-1/2
norms, sorting by (window-group, source-chunk, col), call schedule) happens
on host. int16 gather indices limit tables to <32768 rows, so source tables
are split into 4 chunks.
"""
import os
import sys

for _p in ("/opt/trn_rl_repo",):
    if _p not in sys.path:
        sys.path.insert(0, _p)

import numpy as np
import ml_dtypes


def _install_trace_shim():
    # the agent image's antenv lacks axon_hooks; recreate it so
    # bass_utils trace=True works (profiling only; harmless otherwise)
    import types
    try:
        import antenv
        if "antenv.axon_hooks" in sys.modules:
            return
        mod = types.ModuleType("antenv.axon_hooks")
        _hook = [None]
        mod.set_axon_ntff_profile_hook = lambda h: _hook.__setitem__(0, h)
        mod.get_axon_ntff_profile_hook = lambda: _hook[0]
        sys.modules["antenv.axon_hooks"] = mod
        antenv.axon_hooks = mod
        from trn_agent_boot.trn_boot import _ntff_profile_via_ctypes
        h = _ntff_profile_via_ctypes("/opt/axon/libaxon_pjrt.so")
        if h is not None:
            mod.set_axon_ntff_profile_hook(h)
    except Exception:
        pass


_install_trace_shim()

from concourse import bacc, bass, mybir, tile
from concourse.bass_utils import run_bass_kernel_spmd

BF16 = ml_dtypes.bfloat16

N, E, DIN, H = 100000, 1600000, 128, 128
NCORES = 8
NPC = 12500            # nodes per core
NPC_PAD = 12544        # 98 windows * 128
NWIN = 98
WG_W = 2               # windows per psum group
NWG = NWIN // WG_W     # 49
WGD = WG_W * 128       # 256 dests per group
GATHER_NI = 1024       # max idxs per dma_gather call
PAD_COL = 30000.0      # col sentinel for pad slots (never matches iota)

LAST_EXEC_NS = None
LAST_RESULTS = None


def _ceil(a, b):
    return (a + b - 1) // b


def _plan_layer(ecol_l, esrc_l, enorm_l, tbl_rows):
    """Build the shared call schedule + per-core slot streams for one layer.

    ecol_l[c]: destination (core-local, 0..12499) per edge, per core
    esrc_l[c]: source row in the gather table (global) per edge, or -1 for
        self-loops (routed to table 4, the per-core "own" buffer, idx=ecol)
    enorm_l[c]: edge weight
    tbl_rows: rows per table chunk (tables 0..3)
    """
    ncore = len(ecol_l)
    NT = 5
    per_core = []
    counts = np.zeros((ncore, NWG, NT), np.int64)
    for c in range(ncore):
        ecol, esrc, enorm = ecol_l[c], esrc_l[c], enorm_l[c]
        wg = ecol // WGD
        ch = np.where(esrc < 0, 4, esrc // tbl_rows)
        order = np.argsort(wg * NT + ch, kind="stable")
        ecol, esrc, enorm = (a[order] for a in (ecol, esrc, enorm))
        per_core.append((ecol, esrc, enorm))
        wg, ch = wg[order], ch[order]
        counts[c] = np.bincount(wg * NT + ch,
                                minlength=NWG * NT).reshape(NWG, NT)

    maxcnt = counts.max(axis=0)
    slots = ((maxcnt + 127) // 128) * 128

    sched = []
    slot0 = 0
    for g in range(NWG):
        calls = []
        for ch in range(NT):
            rem = int(slots[g, ch])
            while rem > 0:
                ni = min(GATHER_NI, rem)
                calls.append((ch, ni, slot0))
                slot0 += ni
                rem -= ni
        sched.append(calls)
    total_slots = slot0
    nch = total_slots // 128
    icols = total_slots // 16

    idx16 = np.zeros((ncore, 128, icols), np.int16)
    colrel = np.full((ncore, 128, nch), PAD_COL, np.float32)
    normb = np.zeros((ncore, 128, nch), np.float32)

    sbase = np.zeros(NWG * NT + 1, np.int64)
    sbase[1:] = np.cumsum(slots.reshape(-1))
    for c in range(ncore):
        ecol, esrc, enorm = per_core[c]
        wg = ecol // WGD
        ch = np.where(esrc < 0, 4, esrc // tbl_rows)
        flat = wg * NT + ch
        cnts = counts[c].reshape(-1)
        run_start = np.zeros(NWG * NT, np.int64)
        run_start[1:] = np.cumsum(cnts)[:-1]
        pos_in_run = np.arange(len(ecol)) - run_start[flat]
        slot = sbase[flat] + pos_in_run

        tbl_idx = np.where(esrc < 0, ecol, esrc % tbl_rows)
        sidx = np.zeros(total_slots, np.int16)
        scol = np.full(total_slots, PAD_COL, np.float64)
        snorm = np.zeros(total_slots, np.float64)
        sidx[slot] = tbl_idx.astype(np.int16)
        scol[slot] = (ecol % WGD).astype(np.float64)
        snorm[slot] = enorm
        i16 = sidx.reshape(icols, 16).T
        idx16[c] = np.tile(i16, (8, 1))
        colrel[c] = scol.reshape(nch, 128).T.astype(np.float32)
        normb[c] = snorm.reshape(nch, 128).T.astype(np.float32)

    return sched, total_slots, idx16, colrel, normb


def _emit_layer(nc, tc, pools, sched, tables, idx_in, colrel_in, norm_in,
                iota_t, ones_t, W_t, b_t, hout, h1p_local, dinv_sb):
    """Emit one GCN layer. tables: list of 4 DRAM APs (chunk tables).
    h1p_local: DRAM tile for bf16 output (or None for layer 2)."""
    sb, ip, msk, psum, ep = pools
    for g, calls in enumerate(sched):
        nslots = sum(ni for _, ni, _ in calls)
        nchunks = nslots // 128
        s0 = calls[0][2]
        ch0 = s0 // 128
        msgs = sb.tile([128, nchunks, 128], mybir.dt.bfloat16, name="msgs",
                       tag="msgs")
        for (ch, ni, slot0) in calls:
            b0 = (slot0 - s0) // 128
            it = ip.tile([128, ni // 16], mybir.dt.int16, name="it", tag="it")
            nc.sync.dma_start(out=it[:], in_=idx_in[:, slot0 // 16:
                                                    (slot0 + ni) // 16])
            nc.gpsimd.dma_gather(
                msgs[:, b0:b0 + ni // 128, :], tables[ch], it[:], ni, ni, 128,
                queue_num=(slot0 // GATHER_NI) % 4, single_packet=False)
        cr = ip.tile([128, nchunks], mybir.dt.float32, name="cr", tag="cr")
        nm = ip.tile([128, nchunks], mybir.dt.float32, name="nm", tag="nm")
        nc.sync.dma_start(out=cr[:], in_=colrel_in[:, ch0:ch0 + nchunks])
        nc.sync.dma_start(out=nm[:], in_=norm_in[:, ch0:ch0 + nchunks])

        aggp = psum.tile([128, WGD], mybir.dt.float32, name="aggp",
                         tag="aggp", space="PSUM")
        for j in range(nchunks):
            m = msk.tile([128, WGD], mybir.dt.bfloat16, name="m", tag="m")
            nc.vector.tensor_scalar(
                out=m[:], in0=iota_t[:], scalar1=cr[:, j:j + 1],
                scalar2=nm[:, j:j + 1],
                op0=mybir.AluOpType.is_equal, op1=mybir.AluOpType.mult)
            nc.tensor.matmul(out=aggp[:], lhsT=msgs[:, j, :], rhs=m[:],
                             start=(j == 0), stop=(j == nchunks - 1))
        aggs = ep.tile([128, WGD], mybir.dt.bfloat16, name="aggs", tag="aggs")
        nc.vector.tensor_copy(out=aggs[:], in_=aggp[:])
        for wloc in range(WG_W):
            w = g * WG_W + wloc
            hp = psum.tile([128, H], mybir.dt.float32, name="hp", tag="hp",
                           space="PSUM")
            nc.tensor.matmul(out=hp[:], lhsT=aggs[:, wloc * 128:
                                                  (wloc + 1) * 128],
                             rhs=W_t[:], start=True, stop=False)
            nc.tensor.matmul(out=hp[:], lhsT=ones_t[:], rhs=b_t[:],
                             start=False, stop=True)
            hsb = ep.tile([128, H], mybir.dt.float32, name="hsb", tag="hsb")
            nc.scalar.activation(out=hsb[:], in_=hp[:],
                                 func=mybir.ActivationFunctionType.Relu)
            nc.sync.dma_start(out=hout[w * 128:(w + 1) * 128, :], in_=hsb[:])
            if h1p_local is not None:
                hbf = ep.tile([128, H], mybir.dt.bfloat16, name="hbf",
                              tag="hbf")
                nc.vector.tensor_copy(out=hbf[:], in_=hsb[:])
                nc.sync.dma_start(out=h1p_local[w * 128:(w + 1) * 128, :],
                                  in_=hbf[:])


def _build_nc(sched1, tot1, sched2, tot2):
    nc = bacc.Bacc("TRN2", num_swdge_queues=4)

    # inputs (per-core data under the same names)
    x4 = [nc.declare_dram_parameter(f"x{i}", [N // 4, DIN],
                                    mybir.dt.bfloat16, isOutput=False)
          for i in range(4)]
    x_own = nc.declare_dram_parameter("x_own", [NPC_PAD, DIN],
                                      mybir.dt.bfloat16, isOutput=False)
    idx1 = nc.declare_dram_parameter("idx1", [128, tot1 // 16],
                                     mybir.dt.int16, isOutput=False)
    cr1 = nc.declare_dram_parameter("cr1", [128, tot1 // 128],
                                    mybir.dt.float32, isOutput=False)
    nm1 = nc.declare_dram_parameter("nm1", [128, tot1 // 128],
                                    mybir.dt.float32, isOutput=False)
    idx2 = nc.declare_dram_parameter("idx2", [128, tot2 // 16],
                                     mybir.dt.int16, isOutput=False)
    cr2 = nc.declare_dram_parameter("cr2", [128, tot2 // 128],
                                    mybir.dt.float32, isOutput=False)
    nm2 = nc.declare_dram_parameter("nm2", [128, tot2 // 128],
                                    mybir.dt.float32, isOutput=False)
    Wb1 = nc.declare_dram_parameter("Wb1", [DIN, H], mybir.dt.bfloat16,
                                    isOutput=False)
    Wb2 = nc.declare_dram_parameter("Wb2", [H, H], mybir.dt.bfloat16,
                                    isOutput=False)
    bb = nc.declare_dram_parameter("bb", [2, H], mybir.dt.bfloat16,
                                   isOutput=False)
    iota_in = nc.declare_dram_parameter("iota", [128, WGD], mybir.dt.bfloat16,
                                        isOutput=False)
    h1out = nc.declare_dram_parameter("h1out", [NPC_PAD, H],
                                      mybir.dt.float32, isOutput=True)
    h2out = nc.declare_dram_parameter("h2out", [NPC_PAD, H],
                                      mybir.dt.float32, isOutput=True)

    with tile.TileContext(nc) as tc:
        with tc.tile_pool(name="sb", bufs=3) as sb, \
             tc.tile_pool(name="ip", bufs=6) as ip, \
             tc.tile_pool(name="msk", bufs=6) as msk, \
             tc.tile_pool(name="psum", bufs=2, space="PSUM") as psum, \
             tc.tile_pool(name="ep", bufs=3) as ep, \
             tc.tile_pool(name="const", bufs=1) as cst, \
             tc.tile_pool(name="dram", bufs=1, space="DRAM") as dram:

            h1p_local = dram.tile([NPC_PAD, H], mybir.dt.bfloat16,
                                  name="h1p_local")
            h1p_full = dram.tile([NCORES * NPC_PAD, H], mybir.dt.bfloat16,
                                 name="h1p_full", addr_space="Shared")

            iota_t = cst.tile([128, WGD], mybir.dt.bfloat16, name="iota_t")
            nc.sync.dma_start(out=iota_t[:], in_=iota_in[:])
            ones_t = cst.tile([1, 128], mybir.dt.bfloat16, name="ones_t")
            nc.vector.memset(ones_t[:], 1.0)
            W1_t = cst.tile([DIN, H], mybir.dt.bfloat16, name="W1_t")
            nc.sync.dma_start(out=W1_t[:], in_=Wb1[:])
            W2_t = cst.tile([H, H], mybir.dt.bfloat16, name="W2_t")
            nc.sync.dma_start(out=W2_t[:], in_=Wb2[:])
            b1_t = cst.tile([1, H], mybir.dt.bfloat16, name="b1_t")
            nc.sync.dma_start(out=b1_t[:], in_=bb[0:1, :])
            b2_t = cst.tile([1, H], mybir.dt.bfloat16, name="b2_t")
            nc.sync.dma_start(out=b2_t[:], in_=bb[1:2, :])

            pools = (sb, ip, msk, psum, ep)
            _emit_layer(nc, tc, pools, sched1,
                        [t[:] for t in x4] + [x_own[:]], idx1, cr1, nm1,
                        iota_t, ones_t, W1_t, b1_t, h1out, h1p_local, None)

            nc.gpsimd.collective_compute(
                "AllGather", mybir.AluOpType.bypass,
                replica_groups=[list(range(NCORES))],
                ins=[h1p_local.opt()], outs=[h1p_full.opt()])

            tbl2_rows = NCORES * NPC_PAD // 4
            tables2 = [h1p_full[i * tbl2_rows:(i + 1) * tbl2_rows]
                       for i in range(4)] + [h1p_local[:]]
            _emit_layer(nc, tc, pools, sched2,
                        tables2, idx2, cr2, nm2,
                        iota_t, ones_t, W2_t, b2_t, h2out, None, None)

    nc.finalize()
    return nc


def kernel(x, edge_index, W1, b1, W2, b2):
    global LAST_EXEC_NS, LAST_RESULTS
    x = np.asarray(x, np.float32)
    edge_index = np.asarray(edge_index)
    W1 = np.asarray(W1, np.float32)
    b1 = np.asarray(b1, np.float32)
    W2 = np.asarray(W2, np.float32)
    b2 = np.asarray(b2, np.float32)
    n = x.shape[0]

    row = edge_index[0].astype(np.int64)
    col = edge_index[1].astype(np.int64)
    loop = np.arange(n, dtype=np.int64)
    row_sl = np.concatenate([row, loop])
    col_sl = np.concatenate([col, loop])
    deg = np.bincount(col_sl, minlength=n).astype(np.float64)
    dinv = np.where(deg > 0, 1.0 / np.sqrt(deg), 0.0)
    norm = dinv[row_sl] * dinv[col_sl]

    is_self = np.concatenate([np.zeros(len(row), bool), np.ones(n, bool)])
    core = col_sl // NPC
    ecol_l, esrc1_l, esrc2_l, enorm_l = [], [], [], []
    for c in range(NCORES):
        m = core == c
        sl = is_self[m]
        ecol_l.append(col_sl[m] - c * NPC)
        r = row_sl[m]
        esrc1_l.append(np.where(sl, -1, r))
        # layer-2 source rows live in the allgathered padded layout
        esrc2_l.append(np.where(sl, -1, (r // NPC) * NPC_PAD + (r % NPC)))
        enorm_l.append(norm[m])

    sched1, tot1, idx1, cr1, nm1 = _plan_layer(
        ecol_l, esrc1_l, enorm_l, N // 4)
    sched2, tot2, idx2, cr2, nm2 = _plan_layer(
        ecol_l, esrc2_l, enorm_l, NCORES * NPC_PAD // 4)

    nc = _build_nc(sched1, tot1, sched2, tot2)

    xb = x.astype(BF16)
    Wb1v = W1.astype(BF16)
    Wb2v = W2.astype(BF16)
    bbv = np.stack([b1, b2]).astype(BF16)
    iota = np.tile(np.arange(WGD, dtype=np.float32).astype(BF16), (128, 1))

    in_maps = []
    for c in range(NCORES):
        m = {f"x{i}": xb[i * (N // 4):(i + 1) * (N // 4)] for i in range(4)}
        xo = np.zeros((NPC_PAD, DIN), BF16)
        xo[:NPC] = xb[c * NPC:(c + 1) * NPC]
        m.update(x_own=xo, idx1=idx1[c], cr1=cr1[c], nm1=nm1[c],
                 idx2=idx2[c], cr2=cr2[c], nm2=nm2[c],
                 Wb1=Wb1v, Wb2=Wb2v, bb=bbv, iota=iota)
        in_maps.append(m)

    res = run_bass_kernel_spmd(
        nc, in_maps, core_ids=list(range(NCORES)),
        trace=bool(int(os.environ.get("BASS_TRACE_KERNEL", "0"))))
    LAST_EXEC_NS = res.exec_time_ns
    LAST_RESULTS = res

    h1 = np.concatenate([res.results[c]["h1out"][:NPC] for c in range(NCORES)])
    h2 = np.concatenate([res.results[c]["h2out"][:NPC] for c in range(NCORES)])
    return np.concatenate([h1, h2], axis=1).astype(np.float32)


# revision 15
# speedup vs baseline: 1.4430x; 1.4430x over previous
"""GCN 2-layer encoder on 8 trn2 NeuronCores (Bass/Tile).

Strategy (per sharding hint): partition destination nodes across 8 cores
(12500/core); each layer computes agg = A_norm @ h via per-edge gather
(dma_gather, bf16) + TensorE segment-sum using norm-scaled one-hot masks
built on VectorE, then a [128,128] GEMM (+bias matmul) and relu.
Aggregate-then-transform order: A@(xW) == (A@x)@W. Layer boundary uses an
AllGather of the bf16 h1 slices. Graph preprocessing (self loops, deg^-1/2
norms, sorting by (window-group, source-chunk, col), call schedule) happens
on host. int16 gather indices limit tables to <32768 rows, so source tables
are split into 4 chunks.
"""
import os
import sys

for _p in ("/opt/trn_rl_repo",):
    if _p not in sys.path:
        sys.path.insert(0, _p)

import numpy as np
import ml_dtypes


def _install_trace_shim():
    # the agent image's antenv lacks axon_hooks; recreate it so
    # bass_utils trace=True works (profiling only; harmless otherwise)
    import types
    try:
        import antenv
        if "antenv.axon_hooks" in sys.modules:
            return
        mod = types.ModuleType("antenv.axon_hooks")
        _hook = [None]
        mod.set_axon_ntff_profile_hook = lambda h: _hook.__setitem__(0, h)
        mod.get_axon_ntff_profile_hook = lambda: _hook[0]
        sys.modules["antenv.axon_hooks"] = mod
        antenv.axon_hooks = mod
        from trn_agent_boot.trn_boot import _ntff_profile_via_ctypes
        h = _ntff_profile_via_ctypes("/opt/axon/libaxon_pjrt.so")
        if h is not None:
            mod.set_axon_ntff_profile_hook(h)
    except Exception:
        pass


_install_trace_shim()

from concourse import bacc, bass, mybir, tile
from concourse.bass_utils import run_bass_kernel_spmd

BF16 = ml_dtypes.bfloat16

N, E, DIN, H = 100000, 1600000, 128, 128
NCORES = 8
NPC = 12500            # nodes per core
NPC_PAD = 12544        # 98 windows * 128
NWIN = 98
WG_W = 2               # windows per psum group
NWG = NWIN // WG_W     # 49
WGD = WG_W * 128       # 256 dests per group
GATHER_NI = 1024       # max idxs per dma_gather call
PAD_COL = 30000.0      # col sentinel for pad slots (never matches iota)

LAST_EXEC_NS = None
LAST_RESULTS = None


def _ceil(a, b):
    return (a + b - 1) // b


def _plan_layer(ecol_l, esrc_l, tbl_rows):
    """Build the shared call schedule + per-core slot streams for one layer.

    ecol_l[c]: destination (core-local, 0..12499) per edge, per core
    esrc_l[c]: source row in the gather table (global) per edge, or -1 for
        self-loops (routed to table 4, the per-core "own" buffer, idx=ecol)
    enorm_l[c]: edge weight
    tbl_rows: rows per table chunk (tables 0..3)
    """
    ncore = len(ecol_l)
    NT = 5
    per_core = []
    counts = np.zeros((ncore, NWG, NT), np.int64)
    for c in range(ncore):
        ecol, esrc = ecol_l[c], esrc_l[c]
        wg = ecol // WGD
        ch = np.where(esrc < 0, 4, esrc // tbl_rows)
        order = np.argsort(wg * NT + ch, kind="stable")
        ecol, esrc = (a[order] for a in (ecol, esrc))
        per_core.append((ecol, esrc))
        wg, ch = wg[order], ch[order]
        counts[c] = np.bincount(wg * NT + ch,
                                minlength=NWG * NT).reshape(NWG, NT)

    maxcnt = counts.max(axis=0)
    slots = ((maxcnt + 127) // 128) * 128

    sched = []
    slot0 = 0
    for g in range(NWG):
        calls = []
        for ch in range(NT):
            rem = int(slots[g, ch])
            while rem > 0:
                ni = min(GATHER_NI, rem)
                calls.append((ch, ni, slot0))
                slot0 += ni
                rem -= ni
        sched.append(calls)
    total_slots = slot0
    nch = total_slots // 128
    icols = total_slots // 16

    idx16 = np.zeros((ncore, 128, icols), np.int16)
    colrel = np.full((ncore, 128, nch), PAD_COL, BF16)

    sbase = np.zeros(NWG * NT + 1, np.int64)
    sbase[1:] = np.cumsum(slots.reshape(-1))
    for c in range(ncore):
        ecol, esrc = per_core[c]
        wg = ecol // WGD
        ch = np.where(esrc < 0, 4, esrc // tbl_rows)
        flat = wg * NT + ch
        cnts = counts[c].reshape(-1)
        run_start = np.zeros(NWG * NT, np.int64)
        run_start[1:] = np.cumsum(cnts)[:-1]
        pos_in_run = np.arange(len(ecol)) - run_start[flat]
        slot = sbase[flat] + pos_in_run

        tbl_idx = np.where(esrc < 0, ecol, esrc % tbl_rows)
        sidx = np.zeros(total_slots, np.int16)
        scol = np.full(total_slots, PAD_COL, np.float64)
        sidx[slot] = tbl_idx.astype(np.int16)
        scol[slot] = (ecol % WGD).astype(np.float64)
        i16 = sidx.reshape(icols, 16).T
        idx16[c] = np.tile(i16, (8, 1))
        colrel[c] = scol.reshape(nch, 128).T.astype(BF16)

    return sched, total_slots, idx16, colrel


def _emit_layer(nc, tc, pools, sched, tables, idx_in, colrel_in,
                iota_t, ones_t, W_t, b_t, dinvrep_t, dinvcol_t,
                hout, h1p_local, qctr):
    """Emit one GCN layer. tables: 5 DRAM APs (4 chunk tables + own).
    Sources are host-prescaled by dinv; dest-side dinv applied on the
    PSUM->SBUF copy (agg) so masks are pure one-hots."""
    sb, ip, msk, psum, ep = pools
    for g, calls in enumerate(sched):
        nslots = sum(ni for _, ni, _ in calls)
        nchunks = nslots // 128
        s0 = calls[0][2]
        ch0 = s0 // 128
        msgs = sb.tile([128, nchunks, 128], mybir.dt.bfloat16, name="msgs",
                       tag="msgs")
        it = ip.tile([128, nslots // 16], mybir.dt.int16, name="it", tag="it")
        nc.sync.dma_start(out=it[:], in_=idx_in[:, s0 // 16:
                                                (s0 + nslots) // 16])
        for (ch, ni, slot0) in calls:
            b0 = (slot0 - s0) // 128
            i0 = (slot0 - s0) // 16
            nc.gpsimd.dma_gather(
                msgs[:, b0:b0 + ni // 128, :], tables[ch],
                it[:, i0:i0 + ni // 16], ni, ni, 128,
                queue_num=qctr[0] % 4, single_packet=False)
            qctr[0] += 1
        cr = ip.tile([128, nchunks], mybir.dt.bfloat16, name="cr", tag="cr")
        nc.sync.dma_start(out=cr[:], in_=colrel_in[:, ch0:ch0 + nchunks])

        aggp = psum.tile([128, WGD], mybir.dt.float32, name="aggp",
                         tag="aggp", space="PSUM")
        for j in range(nchunks):
            m = msk.tile([128, WGD], mybir.dt.bfloat16, name="m", tag="m")
            nc.vector.tensor_tensor(
                out=m[:], in0=iota_t[:],
                in1=cr[:, j:j + 1].to_broadcast([128, WGD]),
                op=mybir.AluOpType.is_equal)
            nc.tensor.matmul(out=aggp[:], lhsT=msgs[:, j, :], rhs=m[:],
                             start=(j == 0), stop=(j == nchunks - 1))
        aggs = ep.tile([128, WGD], mybir.dt.bfloat16, name="aggs", tag="aggs")
        nc.vector.tensor_tensor(
            out=aggs[:], in0=aggp[:],
            in1=dinvrep_t[:, g * WGD:(g + 1) * WGD],
            op=mybir.AluOpType.mult)
        for wloc in range(WG_W):
            w = g * WG_W + wloc
            hp = psum.tile([128, H], mybir.dt.float32, name="hp", tag="hp",
                           space="PSUM")
            nc.tensor.matmul(out=hp[:], lhsT=aggs[:, wloc * 128:
                                                  (wloc + 1) * 128],
                             rhs=W_t[:], start=True, stop=False)
            nc.tensor.matmul(out=hp[:], lhsT=ones_t[:], rhs=b_t[:],
                             start=False, stop=True)
            hsb = ep.tile([128, H], mybir.dt.float32, name="hsb", tag="hsb")
            nc.scalar.activation(out=hsb[:], in_=hp[:],
                                 func=mybir.ActivationFunctionType.Relu)
            nc.sync.dma_start(out=hout[w * 128:(w + 1) * 128, :], in_=hsb[:])
            if h1p_local is not None:
                hbf = ep.tile([128, H], mybir.dt.bfloat16, name="hbf",
                              tag="hbf")
                nc.scalar.activation(out=hbf[:], in_=hp[:],
                                     func=mybir.ActivationFunctionType.Relu,
                                     scale=dinvcol_t[:, w:w + 1])
                nc.sync.dma_start(out=h1p_local[w * 128:(w + 1) * 128, :],
                                  in_=hbf[:])


def _build_nc(sched1, tot1, sched2, tot2):
    nc = bacc.Bacc("TRN2", num_swdge_queues=4)

    # inputs (per-core data under the same names)
    x4 = [nc.declare_dram_parameter(f"x{i}", [N // 4, DIN],
                                    mybir.dt.bfloat16, isOutput=False)
          for i in range(4)]
    x_own = nc.declare_dram_parameter("x_own", [NPC_PAD, DIN],
                                      mybir.dt.bfloat16, isOutput=False)
    idx1 = nc.declare_dram_parameter("idx1", [128, tot1 // 16],
                                     mybir.dt.int16, isOutput=False)
    cr1 = nc.declare_dram_parameter("cr1", [128, tot1 // 128],
                                    mybir.dt.bfloat16, isOutput=False)
    idx2 = nc.declare_dram_parameter("idx2", [128, tot2 // 16],
                                     mybir.dt.int16, isOutput=False)
    cr2 = nc.declare_dram_parameter("cr2", [128, tot2 // 128],
                                    mybir.dt.bfloat16, isOutput=False)
    dinvrep = nc.declare_dram_parameter("dinvrep", [128, NPC_PAD],
                                        mybir.dt.float32, isOutput=False)
    dinvcol = nc.declare_dram_parameter("dinvcol", [128, NWIN],
                                        mybir.dt.float32, isOutput=False)
    Wb1 = nc.declare_dram_parameter("Wb1", [DIN, H], mybir.dt.bfloat16,
                                    isOutput=False)
    Wb2 = nc.declare_dram_parameter("Wb2", [H, H], mybir.dt.bfloat16,
                                    isOutput=False)
    bb = nc.declare_dram_parameter("bb", [2, H], mybir.dt.bfloat16,
                                   isOutput=False)
    iota_in = nc.declare_dram_parameter("iota", [128, WGD], mybir.dt.bfloat16,
                                        isOutput=False)
    h1out = nc.declare_dram_parameter("h1out", [NPC_PAD, H],
                                      mybir.dt.float32, isOutput=True)
    h2out = nc.declare_dram_parameter("h2out", [NPC_PAD, H],
                                      mybir.dt.float32, isOutput=True)

    with tile.TileContext(nc) as tc:
        with tc.tile_pool(name="sb", bufs=3) as sb, \
             tc.tile_pool(name="ip", bufs=6) as ip, \
             tc.tile_pool(name="msk", bufs=6) as msk, \
             tc.tile_pool(name="psum", bufs=2, space="PSUM") as psum, \
             tc.tile_pool(name="ep", bufs=3) as ep, \
             tc.tile_pool(name="const", bufs=1) as cst, \
             tc.tile_pool(name="dram", bufs=1, space="DRAM") as dram:

            h1p_local = dram.tile([NPC_PAD, H], mybir.dt.bfloat16,
                                  name="h1p_local")
            h1p_full = dram.tile([NCORES * NPC_PAD, H], mybir.dt.bfloat16,
                                 name="h1p_full", addr_space="Shared")

            iota_t = cst.tile([128, WGD], mybir.dt.bfloat16, name="iota_t")
            nc.sync.dma_start(out=iota_t[:], in_=iota_in[:])
            ones_t = cst.tile([1, 128], mybir.dt.bfloat16, name="ones_t")
            nc.vector.memset(ones_t[:], 1.0)
            W1_t = cst.tile([DIN, H], mybir.dt.bfloat16, name="W1_t")
            nc.sync.dma_start(out=W1_t[:], in_=Wb1[:])
            W2_t = cst.tile([H, H], mybir.dt.bfloat16, name="W2_t")
            nc.sync.dma_start(out=W2_t[:], in_=Wb2[:])
            b1_t = cst.tile([1, H], mybir.dt.bfloat16, name="b1_t")
            nc.sync.dma_start(out=b1_t[:], in_=bb[0:1, :])
            b2_t = cst.tile([1, H], mybir.dt.bfloat16, name="b2_t")
            nc.sync.dma_start(out=b2_t[:], in_=bb[1:2, :])
            dinvrep_t = cst.tile([128, NPC_PAD], mybir.dt.float32,
                                 name="dinvrep_t")
            nc.sync.dma_start(out=dinvrep_t[:], in_=dinvrep[:])
            dinvcol_t = cst.tile([128, NWIN], mybir.dt.float32,
                                 name="dinvcol_t")
            nc.sync.dma_start(out=dinvcol_t[:], in_=dinvcol[:])

            pools = (sb, ip, msk, psum, ep)
            qctr = [0]
            _emit_layer(nc, tc, pools, sched1,
                        [t[:] for t in x4] + [x_own[:]], idx1, cr1,
                        iota_t, ones_t, W1_t, b1_t, dinvrep_t, dinvcol_t,
                        h1out, h1p_local, qctr)

            nc.gpsimd.collective_compute(
                "AllGather", mybir.AluOpType.bypass,
                replica_groups=[list(range(NCORES))],
                ins=[h1p_local.opt()], outs=[h1p_full.opt()])

            tbl2_rows = NCORES * NPC_PAD // 4
            tables2 = [h1p_full[i * tbl2_rows:(i + 1) * tbl2_rows]
                       for i in range(4)] + [h1p_local[:]]
            _emit_layer(nc, tc, pools, sched2,
                        tables2, idx2, cr2,
                        iota_t, ones_t, W2_t, b2_t, dinvrep_t, dinvcol_t,
                        h2out, None, qctr)

    nc.finalize()
    return nc


def kernel(x, edge_index, W1, b1, W2, b2):
    global LAST_EXEC_NS, LAST_RESULTS
    x = np.asarray(x, np.float32)
    edge_index = np.asarray(edge_index)
    W1 = np.asarray(W1, np.float32)
    b1 = np.asarray(b1, np.float32)
    W2 = np.asarray(W2, np.float32)
    b2 = np.asarray(b2, np.float32)
    n = x.shape[0]

    row = edge_index[0].astype(np.int64)
    col = edge_index[1].astype(np.int64)
    loop = np.arange(n, dtype=np.int64)
    row_sl = np.concatenate([row, loop])
    col_sl = np.concatenate([col, loop])
    deg = np.bincount(col_sl, minlength=n).astype(np.float64)
    dinv = np.where(deg > 0, 1.0 / np.sqrt(deg), 0.0).astype(np.float32)

    is_self = np.concatenate([np.zeros(len(row), bool), np.ones(n, bool)])
    core = col_sl // NPC
    ecol_l, esrc1_l, esrc2_l = [], [], []
    for c in range(NCORES):
        m = core == c
        sl = is_self[m]
        ecol_l.append(col_sl[m] - c * NPC)
        r = row_sl[m]
        esrc1_l.append(np.where(sl, -1, r))
        # layer-2 source rows live in the allgathered padded layout
        esrc2_l.append(np.where(sl, -1, (r // NPC) * NPC_PAD + (r % NPC)))

    sched1, tot1, idx1, cr1 = _plan_layer(ecol_l, esrc1_l, N // 4)
    sched2, tot2, idx2, cr2 = _plan_layer(ecol_l, esrc2_l,
                                          NCORES * NPC_PAD // 4)

    nc = _build_nc(sched1, tot1, sched2, tot2)

    xb = (x * dinv[:, None].astype(np.float32)).astype(BF16)
    Wb1v = W1.astype(BF16)
    Wb2v = W2.astype(BF16)
    bbv = np.stack([b1, b2]).astype(BF16)
    iota = np.tile(np.arange(WGD, dtype=np.float32).astype(BF16), (128, 1))

    in_maps = []
    for c in range(NCORES):
        m = {f"x{i}": xb[i * (N // 4):(i + 1) * (N // 4)] for i in range(4)}
        xo = np.zeros((NPC_PAD, DIN), BF16)
        xo[:NPC] = xb[c * NPC:(c + 1) * NPC]
        dloc = np.zeros(NPC_PAD, np.float32)
        dloc[:NPC] = dinv[c * NPC:(c + 1) * NPC]
        m.update(x_own=xo, idx1=idx1[c], cr1=cr1[c],
                 idx2=idx2[c], cr2=cr2[c],
                 dinvrep=np.tile(dloc, (128, 1)),
                 dinvcol=np.ascontiguousarray(
                     dloc.reshape(NWIN, 128).T),
                 Wb1=Wb1v, Wb2=Wb2v, bb=bbv, iota=iota)
        in_maps.append(m)

    res = run_bass_kernel_spmd(
        nc, in_maps, core_ids=list(range(NCORES)),
        trace=bool(int(os.environ.get("BASS_TRACE_KERNEL", "0"))))
    LAST_EXEC_NS = res.exec_time_ns
    LAST_RESULTS = res

    h1 = np.concatenate([res.results[c]["h1out"][:NPC] for c in range(NCORES)])
    h2 = np.concatenate([res.results[c]["h2out"][:NPC] for c in range(NCORES)])
    return np.concatenate([h1, h2], axis=1).astype(np.float32)


# revision 16
# speedup vs baseline: 1.7814x; 1.2346x over previous
"""GCN 2-layer encoder on 8 trn2 NeuronCores (Bass/Tile).

Strategy (per sharding hint): partition destination nodes across 8 cores
(12500/core); each layer computes agg = A_norm @ h via per-edge gather
(dma_gather, bf16) + TensorE segment-sum using norm-scaled one-hot masks
built on VectorE, then a [128,128] GEMM (+bias matmul) and relu.
Aggregate-then-transform order: A@(xW) == (A@x)@W. Layer boundary uses an
AllGather of the bf16 h1 slices. Graph preprocessing (self loops, deg^-1/2
norms, sorting by (window-group, source-chunk, col), call schedule) happens
on host. int16 gather indices limit tables to <32768 rows, so source tables
are split into 4 chunks.
"""
import os
import sys

for _p in ("/opt/trn_rl_repo",):
    if _p not in sys.path:
        sys.path.insert(0, _p)

import numpy as np
import ml_dtypes


def _install_trace_shim():
    # the agent image's antenv lacks axon_hooks; recreate it so
    # bass_utils trace=True works (profiling only; harmless otherwise)
    import types
    try:
        import antenv
        if "antenv.axon_hooks" in sys.modules:
            return
        mod = types.ModuleType("antenv.axon_hooks")
        _hook = [None]
        mod.set_axon_ntff_profile_hook = lambda h: _hook.__setitem__(0, h)
        mod.get_axon_ntff_profile_hook = lambda: _hook[0]
        sys.modules["antenv.axon_hooks"] = mod
        antenv.axon_hooks = mod
        from trn_agent_boot.trn_boot import _ntff_profile_via_ctypes
        h = _ntff_profile_via_ctypes("/opt/axon/libaxon_pjrt.so")
        if h is not None:
            mod.set_axon_ntff_profile_hook(h)
    except Exception:
        pass


_install_trace_shim()

from concourse import bacc, bass, mybir, tile
from concourse.bass_utils import run_bass_kernel_spmd

BF16 = ml_dtypes.bfloat16

N, E, DIN, H = 100000, 1600000, 128, 128
NCORES = 8
NPC = 12500            # nodes per core
NPC_PAD = 12544        # 98 windows * 128
NWIN = 98
WG_W = 2               # windows per psum group
NWG = NWIN // WG_W     # 49
WGD = WG_W * 128       # 256 dests per group
GATHER_NI = 1024       # max idxs per dma_gather call
PAD_COL = 30000.0      # col sentinel for pad slots (never matches iota)

LAST_EXEC_NS = None
LAST_RESULTS = None


def _ceil(a, b):
    return (a + b - 1) // b


def _plan_layer(ecol_l, esrc_l, tbl_rows):
    """Build the shared call schedule + per-core slot streams for one layer.

    ecol_l[c]: destination (core-local, 0..12499) per edge, per core
    esrc_l[c]: source row in the gather table (global) per edge, or -1 for
        self-loops (routed to table 4, the per-core "own" buffer, idx=ecol)
    enorm_l[c]: edge weight
    tbl_rows: rows per table chunk (tables 0..3)
    """
    ncore = len(ecol_l)
    NT = 5
    per_core = []
    counts = np.zeros((ncore, NWG, NT), np.int64)
    for c in range(ncore):
        ecol, esrc = ecol_l[c], esrc_l[c]
        wg = ecol // WGD
        ch = np.where(esrc < 0, 4, esrc // tbl_rows)
        order = np.argsort(wg * NT + ch, kind="stable")
        ecol, esrc = (a[order] for a in (ecol, esrc))
        per_core.append((ecol, esrc))
        wg, ch = wg[order], ch[order]
        counts[c] = np.bincount(wg * NT + ch,
                                minlength=NWG * NT).reshape(NWG, NT)

    maxcnt = counts.max(axis=0)
    slots = ((maxcnt + 127) // 128) * 128

    sched = []
    slot0 = 0
    for g in range(NWG):
        calls = []
        for ch in range(NT):
            rem = int(slots[g, ch])
            while rem > 0:
                ni = min(GATHER_NI, rem)
                calls.append((ch, ni, slot0))
                slot0 += ni
                rem -= ni
        sched.append(calls)
    total_slots = slot0
    nch = total_slots // 128
    icols = total_slots // 16

    idx16 = np.zeros((ncore, 128, icols), np.int16)
    colrel = np.full((ncore, 128, nch), PAD_COL, BF16)

    sbase = np.zeros(NWG * NT + 1, np.int64)
    sbase[1:] = np.cumsum(slots.reshape(-1))
    for c in range(ncore):
        ecol, esrc = per_core[c]
        wg = ecol // WGD
        ch = np.where(esrc < 0, 4, esrc // tbl_rows)
        flat = wg * NT + ch
        cnts = counts[c].reshape(-1)
        run_start = np.zeros(NWG * NT, np.int64)
        run_start[1:] = np.cumsum(cnts)[:-1]
        pos_in_run = np.arange(len(ecol)) - run_start[flat]
        slot = sbase[flat] + pos_in_run

        tbl_idx = np.where(esrc < 0, ecol, esrc % tbl_rows)
        sidx = np.zeros(total_slots, np.int16)
        scol = np.full(total_slots, PAD_COL, np.float64)
        sidx[slot] = tbl_idx.astype(np.int16)
        scol[slot] = (ecol % WGD).astype(np.float64)
        i16 = sidx.reshape(icols, 16).T
        idx16[c] = np.tile(i16, (8, 1))
        colrel[c] = scol.reshape(nch, 128).T.astype(BF16)

    return sched, total_slots, idx16, colrel


def _emit_layer(nc, tc, pools, sched, tables, idx_in, colrel_in,
                iota_t, ones_t, W_t, b_t, dinvrep_t, dinvcol_t,
                hout, h1p_local, qctr):
    """Emit one GCN layer. tables: 5 DRAM APs (4 chunk tables + own).
    Sources are host-prescaled by dinv; dest-side dinv applied on the
    PSUM->SBUF copy (agg) so masks are pure one-hots."""
    sb, ip, msk, psum, ep = pools
    for g, calls in enumerate(sched):
        nslots = sum(ni for _, ni, _ in calls)
        nchunks = nslots // 128
        s0 = calls[0][2]
        ch0 = s0 // 128
        msgs = sb.tile([128, nchunks, 128], mybir.dt.bfloat16, name="msgs",
                       tag="msgs")
        it = ip.tile([128, nslots // 16], mybir.dt.int16, name="it", tag="it")
        nc.sync.dma_start(out=it[:], in_=idx_in[:, s0 // 16:
                                                (s0 + nslots) // 16])
        for (ch, ni, slot0) in calls:
            b0 = (slot0 - s0) // 128
            i0 = (slot0 - s0) // 16
            nc.gpsimd.dma_gather(
                msgs[:, b0:b0 + ni // 128, :], tables[ch],
                it[:, i0:i0 + ni // 16], ni, ni, 128,
                queue_num=qctr[0] % 4, single_packet=False)
            qctr[0] += 1
        cr = ip.tile([128, nchunks], mybir.dt.bfloat16, name="cr", tag="cr")
        nc.sync.dma_start(out=cr[:], in_=colrel_in[:, ch0:ch0 + nchunks])

        aggp = psum.tile([128, WGD], mybir.dt.float32, name="aggp",
                         tag="aggp", space="PSUM")
        MB = 8
        for j0 in range(0, nchunks, MB):
            nb = min(MB, nchunks - j0)
            m = msk.tile([128, MB, WGD], mybir.dt.bfloat16, name="m", tag="m")
            nc.vector.tensor_tensor(
                out=m[:, :nb, :], in0=iota_t[:, :nb * WGD],
                in1=cr[:, j0:j0 + nb].unsqueeze(2).to_broadcast(
                    [128, nb, WGD]),
                op=mybir.AluOpType.is_equal)
            for k in range(nb):
                j = j0 + k
                nc.tensor.matmul(out=aggp[:], lhsT=msgs[:, j, :],
                                 rhs=m[:, k, :], start=(j == 0),
                                 stop=(j == nchunks - 1))
        aggs = ep.tile([128, WGD], mybir.dt.bfloat16, name="aggs", tag="aggs")
        nc.vector.tensor_tensor(
            out=aggs[:], in0=aggp[:],
            in1=dinvrep_t[:, g * WGD:(g + 1) * WGD],
            op=mybir.AluOpType.mult)
        for wloc in range(WG_W):
            w = g * WG_W + wloc
            hp = psum.tile([128, H], mybir.dt.float32, name="hp", tag="hp",
                           space="PSUM")
            nc.tensor.matmul(out=hp[:], lhsT=aggs[:, wloc * 128:
                                                  (wloc + 1) * 128],
                             rhs=W_t[:], start=True, stop=False)
            nc.tensor.matmul(out=hp[:], lhsT=ones_t[:], rhs=b_t[:],
                             start=False, stop=True)
            hsb = ep.tile([128, H], mybir.dt.bfloat16, name="hsb",
                          tag="hsb")
            nc.scalar.activation(out=hsb[:], in_=hp[:],
                                 func=mybir.ActivationFunctionType.Relu)
            nc.sync.dma_start(out=hout[w * 128:(w + 1) * 128, :], in_=hsb[:])
            if h1p_local is not None:
                hbf = ep.tile([128, H], mybir.dt.bfloat16, name="hbf",
                              tag="hbf")
                nc.scalar.activation(out=hbf[:], in_=hp[:],
                                     func=mybir.ActivationFunctionType.Relu,
                                     scale=dinvcol_t[:, w:w + 1])
                nc.sync.dma_start(out=h1p_local[w * 128:(w + 1) * 128, :],
                                  in_=hbf[:])


def _build_nc(sched1, tot1, sched2, tot2):
    nc = bacc.Bacc("TRN2", num_swdge_queues=4)

    # inputs (per-core data under the same names)
    x4 = [nc.declare_dram_parameter(f"x{i}", [N // 4, DIN],
                                    mybir.dt.bfloat16, isOutput=False)
          for i in range(4)]
    x_own = nc.declare_dram_parameter("x_own", [NPC_PAD, DIN],
                                      mybir.dt.bfloat16, isOutput=False)
    idx1 = nc.declare_dram_parameter("idx1", [128, tot1 // 16],
                                     mybir.dt.int16, isOutput=False)
    cr1 = nc.declare_dram_parameter("cr1", [128, tot1 // 128],
                                    mybir.dt.bfloat16, isOutput=False)
    idx2 = nc.declare_dram_parameter("idx2", [128, tot2 // 16],
                                     mybir.dt.int16, isOutput=False)
    cr2 = nc.declare_dram_parameter("cr2", [128, tot2 // 128],
                                    mybir.dt.bfloat16, isOutput=False)
    dinvrep = nc.declare_dram_parameter("dinvrep", [128, NPC_PAD],
                                        mybir.dt.float32, isOutput=False)
    dinvcol = nc.declare_dram_parameter("dinvcol", [128, NWIN],
                                        mybir.dt.float32, isOutput=False)
    Wb1 = nc.declare_dram_parameter("Wb1", [DIN, H], mybir.dt.bfloat16,
                                    isOutput=False)
    Wb2 = nc.declare_dram_parameter("Wb2", [H, H], mybir.dt.bfloat16,
                                    isOutput=False)
    bb = nc.declare_dram_parameter("bb", [2, H], mybir.dt.bfloat16,
                                   isOutput=False)
    iota_in = nc.declare_dram_parameter("iota", [128, 8 * WGD],
                                        mybir.dt.bfloat16, isOutput=False)
    h1out = nc.declare_dram_parameter("h1out", [NPC_PAD, H],
                                      mybir.dt.bfloat16, isOutput=True)
    h2out = nc.declare_dram_parameter("h2out", [NPC_PAD, H],
                                      mybir.dt.bfloat16, isOutput=True)

    with tile.TileContext(nc) as tc:
        with tc.tile_pool(name="sb", bufs=3) as sb, \
             tc.tile_pool(name="ip", bufs=6) as ip, \
             tc.tile_pool(name="msk", bufs=6) as msk, \
             tc.tile_pool(name="psum", bufs=2, space="PSUM") as psum, \
             tc.tile_pool(name="ep", bufs=3) as ep, \
             tc.tile_pool(name="const", bufs=1) as cst, \
             tc.tile_pool(name="dram", bufs=1, space="DRAM") as dram:

            h1p_local = dram.tile([NPC_PAD, H], mybir.dt.bfloat16,
                                  name="h1p_local")
            h1p_full = dram.tile([NCORES * NPC_PAD, H], mybir.dt.bfloat16,
                                 name="h1p_full", addr_space="Shared")

            iota_t = cst.tile([128, 8 * WGD], mybir.dt.bfloat16,
                              name="iota_t")
            nc.sync.dma_start(out=iota_t[:], in_=iota_in[:])
            ones_t = cst.tile([1, 128], mybir.dt.bfloat16, name="ones_t")
            nc.vector.memset(ones_t[:], 1.0)
            W1_t = cst.tile([DIN, H], mybir.dt.bfloat16, name="W1_t")
            nc.sync.dma_start(out=W1_t[:], in_=Wb1[:])
            W2_t = cst.tile([H, H], mybir.dt.bfloat16, name="W2_t")
            nc.sync.dma_start(out=W2_t[:], in_=Wb2[:])
            b1_t = cst.tile([1, H], mybir.dt.bfloat16, name="b1_t")
            nc.sync.dma_start(out=b1_t[:], in_=bb[0:1, :])
            b2_t = cst.tile([1, H], mybir.dt.bfloat16, name="b2_t")
            nc.sync.dma_start(out=b2_t[:], in_=bb[1:2, :])
            dinvrep_t = cst.tile([128, NPC_PAD], mybir.dt.float32,
                                 name="dinvrep_t")
            nc.sync.dma_start(out=dinvrep_t[:], in_=dinvrep[:])
            dinvcol_t = cst.tile([128, NWIN], mybir.dt.float32,
                                 name="dinvcol_t")
            nc.sync.dma_start(out=dinvcol_t[:], in_=dinvcol[:])

            pools = (sb, ip, msk, psum, ep)
            qctr = [0]
            _emit_layer(nc, tc, pools, sched1,
                        [t[:] for t in x4] + [x_own[:]], idx1, cr1,
                        iota_t, ones_t, W1_t, b1_t, dinvrep_t, dinvcol_t,
                        h1out, h1p_local, qctr)

            nc.gpsimd.collective_compute(
                "AllGather", mybir.AluOpType.bypass,
                replica_groups=[list(range(NCORES))],
                ins=[h1p_local.opt()], outs=[h1p_full.opt()])

            tbl2_rows = NCORES * NPC_PAD // 4
            tables2 = [h1p_full[i * tbl2_rows:(i + 1) * tbl2_rows]
                       for i in range(4)] + [h1p_local[:]]
            _emit_layer(nc, tc, pools, sched2,
                        tables2, idx2, cr2,
                        iota_t, ones_t, W2_t, b2_t, dinvrep_t, dinvcol_t,
                        h2out, None, qctr)

    nc.finalize()
    return nc


def kernel(x, edge_index, W1, b1, W2, b2):
    global LAST_EXEC_NS, LAST_RESULTS
    x = np.asarray(x, np.float32)
    edge_index = np.asarray(edge_index)
    W1 = np.asarray(W1, np.float32)
    b1 = np.asarray(b1, np.float32)
    W2 = np.asarray(W2, np.float32)
    b2 = np.asarray(b2, np.float32)
    n = x.shape[0]

    row = edge_index[0].astype(np.int64)
    col = edge_index[1].astype(np.int64)
    loop = np.arange(n, dtype=np.int64)
    row_sl = np.concatenate([row, loop])
    col_sl = np.concatenate([col, loop])
    deg = np.bincount(col_sl, minlength=n).astype(np.float64)
    dinv = np.where(deg > 0, 1.0 / np.sqrt(deg), 0.0).astype(np.float32)

    is_self = np.concatenate([np.zeros(len(row), bool), np.ones(n, bool)])
    core = col_sl // NPC
    ecol_l, esrc1_l, esrc2_l = [], [], []
    for c in range(NCORES):
        m = core == c
        sl = is_self[m]
        ecol_l.append(col_sl[m] - c * NPC)
        r = row_sl[m]
        esrc1_l.append(np.where(sl, -1, r))
        # layer-2 source rows live in the allgathered padded layout
        esrc2_l.append(np.where(sl, -1, (r // NPC) * NPC_PAD + (r % NPC)))

    sched1, tot1, idx1, cr1 = _plan_layer(ecol_l, esrc1_l, N // 4)
    sched2, tot2, idx2, cr2 = _plan_layer(ecol_l, esrc2_l,
                                          NCORES * NPC_PAD // 4)

    nc = _build_nc(sched1, tot1, sched2, tot2)

    xb = (x * dinv[:, None].astype(np.float32)).astype(BF16)
    Wb1v = W1.astype(BF16)
    Wb2v = W2.astype(BF16)
    bbv = np.stack([b1, b2]).astype(BF16)
    iota = np.tile(np.arange(WGD, dtype=np.float32).astype(BF16), (128, 8))

    in_maps = []
    for c in range(NCORES):
        m = {f"x{i}": xb[i * (N // 4):(i + 1) * (N // 4)] for i in range(4)}
        xo = np.zeros((NPC_PAD, DIN), BF16)
        xo[:NPC] = xb[c * NPC:(c + 1) * NPC]
        dloc = np.zeros(NPC_PAD, np.float32)
        dloc[:NPC] = dinv[c * NPC:(c + 1) * NPC]
        m.update(x_own=xo, idx1=idx1[c], cr1=cr1[c],
                 idx2=idx2[c], cr2=cr2[c],
                 dinvrep=np.tile(dloc, (128, 1)),
                 dinvcol=np.ascontiguousarray(
                     dloc.reshape(NWIN, 128).T),
                 Wb1=Wb1v, Wb2=Wb2v, bb=bbv, iota=iota)
        in_maps.append(m)

    res = run_bass_kernel_spmd(
        nc, in_maps, core_ids=list(range(NCORES)),
        trace=bool(int(os.environ.get("BASS_TRACE_KERNEL", "0"))))
    LAST_EXEC_NS = res.exec_time_ns
    LAST_RESULTS = res

    h1 = np.concatenate([res.results[c]["h1out"][:NPC].astype(np.float32)
                         for c in range(NCORES)])
    h2 = np.concatenate([res.results[c]["h2out"][:NPC].astype(np.float32)
                         for c in range(NCORES)])
    return np.concatenate([h1, h2], axis=1).astype(np.float32)


# revision 17
# speedup vs baseline: 2.2077x; 1.2393x over previous
"""GCN 2-layer encoder on 8 trn2 NeuronCores (Bass/Tile).

Strategy (per sharding hint): partition destination nodes across 8 cores
(12500/core); each layer computes agg = A_norm @ h via per-edge gather
(dma_gather, bf16) + TensorE segment-sum using norm-scaled one-hot masks
built on VectorE, then a [128,128] GEMM (+bias matmul) and relu.
Aggregate-then-transform order: A@(xW) == (A@x)@W. Layer boundary uses an
AllGather of the bf16 h1 slices. Graph preprocessing (self loops, deg^-1/2
norms, sorting by (window-group, source-chunk, col), call schedule) happens
on host. int16 gather indices limit tables to <32768 rows, so source tables
are split into 4 chunks.
"""
import os
import sys

for _p in ("/opt/trn_rl_repo",):
    if _p not in sys.path:
        sys.path.insert(0, _p)

import numpy as np
import ml_dtypes


def _install_trace_shim():
    # the agent image's antenv lacks axon_hooks; recreate it so
    # bass_utils trace=True works (profiling only; harmless otherwise)
    import types
    try:
        import antenv
        if "antenv.axon_hooks" in sys.modules:
            return
        mod = types.ModuleType("antenv.axon_hooks")
        _hook = [None]
        mod.set_axon_ntff_profile_hook = lambda h: _hook.__setitem__(0, h)
        mod.get_axon_ntff_profile_hook = lambda: _hook[0]
        sys.modules["antenv.axon_hooks"] = mod
        antenv.axon_hooks = mod
        from trn_agent_boot.trn_boot import _ntff_profile_via_ctypes
        h = _ntff_profile_via_ctypes("/opt/axon/libaxon_pjrt.so")
        if h is not None:
            mod.set_axon_ntff_profile_hook(h)
    except Exception:
        pass


_install_trace_shim()

from concourse import bacc, bass, mybir, tile
from concourse.bass_utils import run_bass_kernel_spmd

BF16 = ml_dtypes.bfloat16

N, E, DIN, H = 100000, 1600000, 128, 128
NCORES = 8
NPC = 12500            # nodes per core
NPC_PAD = 12544        # 98 windows * 128
NWIN = 98
WG_W = 2               # windows per psum group
NWG = NWIN // WG_W     # 49
WGD = WG_W * 128       # 256 dests per group
GATHER_NI = 1024       # max idxs per dma_gather call
PAD_COL = 30000.0      # col sentinel for pad slots (never matches iota)

LAST_EXEC_NS = None
LAST_RESULTS = None


def _ceil(a, b):
    return (a + b - 1) // b


def _plan_layer(ecol_l, esrc_l, tbl_rows):
    """Build the shared call schedule + per-core slot streams for one layer.

    ecol_l[c]: destination (core-local, 0..12499) per edge, per core
    esrc_l[c]: source row in the gather table (global) per edge, or -1 for
        self-loops (routed to table 4, the per-core "own" buffer, idx=ecol)
    enorm_l[c]: edge weight
    tbl_rows: rows per table chunk (tables 0..3)
    """
    ncore = len(ecol_l)
    NT = 5
    per_core = []
    counts = np.zeros((ncore, NWG, NT), np.int64)
    for c in range(ncore):
        ecol, esrc = ecol_l[c], esrc_l[c]
        wg = ecol // WGD
        ch = np.where(esrc < 0, 4, esrc // tbl_rows)
        order = np.argsort(wg * NT + ch, kind="stable")
        ecol, esrc = (a[order] for a in (ecol, esrc))
        per_core.append((ecol, esrc))
        wg, ch = wg[order], ch[order]
        counts[c] = np.bincount(wg * NT + ch,
                                minlength=NWG * NT).reshape(NWG, NT)

    maxcnt = counts.max(axis=0)
    slots = ((maxcnt + 127) // 128) * 128

    sched = []
    slot0 = 0
    for g in range(NWG):
        calls = []
        for ch in range(NT):
            rem = int(slots[g, ch])
            ncall = max(1, -(-rem // GATHER_NI))
            base = rem // ncall // 128 * 128
            sizes = [base] * ncall
            sizes[-1] = rem - base * (ncall - 1)
            for ni in sizes:
                if ni <= 0:
                    continue
                calls.append((ch, ni, slot0))
                slot0 += ni
        sched.append(calls)
    total_slots = slot0
    nch = total_slots // 128
    icols = total_slots // 16

    idx16 = np.zeros((ncore, 128, icols), np.int16)
    colrel = np.full((ncore, 128, nch), PAD_COL, BF16)

    sbase = np.zeros(NWG * NT + 1, np.int64)
    sbase[1:] = np.cumsum(slots.reshape(-1))
    for c in range(ncore):
        ecol, esrc = per_core[c]
        wg = ecol // WGD
        ch = np.where(esrc < 0, 4, esrc // tbl_rows)
        flat = wg * NT + ch
        cnts = counts[c].reshape(-1)
        run_start = np.zeros(NWG * NT, np.int64)
        run_start[1:] = np.cumsum(cnts)[:-1]
        pos_in_run = np.arange(len(ecol)) - run_start[flat]
        slot = sbase[flat] + pos_in_run

        tbl_idx = np.where(esrc < 0, ecol, esrc % tbl_rows)
        sidx = np.zeros(total_slots, np.int16)
        scol = np.full(total_slots, PAD_COL, np.float64)
        sidx[slot] = tbl_idx.astype(np.int16)
        scol[slot] = (ecol % WGD).astype(np.float64)
        i16 = sidx.reshape(icols, 16).T
        idx16[c] = np.tile(i16, (8, 1))
        colrel[c] = scol.reshape(nch, 128).T.astype(BF16)

    return sched, total_slots, idx16, colrel


def _emit_layer(nc, tc, pools, sched, tables, idx_in, colrel_in,
                iota_t, ones_t, W_t, b_t, dinvrep_t, dinvcol_t,
                hout, h1p_local, qctr):
    """Emit one GCN layer. tables: 5 DRAM APs (4 chunk tables + own).
    Sources are host-prescaled by dinv; dest-side dinv applied on the
    PSUM->SBUF copy (agg) so masks are pure one-hots."""
    sb, ip, msk, psum, ep = pools
    for g, calls in enumerate(sched):
        nslots = sum(ni for _, ni, _ in calls)
        nchunks = nslots // 128
        s0 = calls[0][2]
        ch0 = s0 // 128
        msgs = sb.tile([128, nchunks, 128], mybir.dt.bfloat16, name="msgs",
                       tag="msgs")
        it = ip.tile([128, nslots // 16], mybir.dt.int16, name="it", tag="it")
        nc.sync.dma_start(out=it[:], in_=idx_in[:, s0 // 16:
                                                (s0 + nslots) // 16])
        for (ch, ni, slot0) in calls:
            b0 = (slot0 - s0) // 128
            i0 = (slot0 - s0) // 16
            nc.gpsimd.dma_gather(
                msgs[:, b0:b0 + ni // 128, :], tables[ch],
                it[:, i0:i0 + ni // 16], ni, ni, 128,
                queue_num=qctr[0] % 4, single_packet=False)
            qctr[0] += 1
        cr = ip.tile([128, nchunks], mybir.dt.bfloat16, name="cr", tag="cr")
        nc.sync.dma_start(out=cr[:], in_=colrel_in[:, ch0:ch0 + nchunks])

        aggp = psum.tile([128, WGD], mybir.dt.float32, name="aggp",
                         tag="aggp", space="PSUM")
        MB = 8
        for j0 in range(0, nchunks, MB):
            nb = min(MB, nchunks - j0)
            m = msk.tile([128, MB, WGD], mybir.dt.bfloat16, name="m", tag="m")
            nc.vector.tensor_tensor(
                out=m[:, :nb, :], in0=iota_t[:, :nb * WGD],
                in1=cr[:, j0:j0 + nb].unsqueeze(2).to_broadcast(
                    [128, nb, WGD]),
                op=mybir.AluOpType.is_equal)
            for k in range(nb):
                j = j0 + k
                nc.tensor.matmul(out=aggp[:], lhsT=msgs[:, j, :],
                                 rhs=m[:, k, :], start=(j == 0),
                                 stop=(j == nchunks - 1))
        aggs = ep.tile([128, WGD], mybir.dt.bfloat16, name="aggs", tag="aggs")
        nc.vector.tensor_tensor(
            out=aggs[:], in0=aggp[:],
            in1=dinvrep_t[:, g * WGD:(g + 1) * WGD],
            op=mybir.AluOpType.mult)
        for wloc in range(WG_W):
            w = g * WG_W + wloc
            hp = psum.tile([128, H], mybir.dt.float32, name="hp", tag="hp",
                           space="PSUM")
            nc.tensor.matmul(out=hp[:], lhsT=aggs[:, wloc * 128:
                                                  (wloc + 1) * 128],
                             rhs=W_t[:], start=True, stop=False)
            nc.tensor.matmul(out=hp[:], lhsT=ones_t[:], rhs=b_t[:],
                             start=False, stop=True)
            hsb = ep.tile([128, H], mybir.dt.bfloat16, name="hsb",
                          tag="hsb")
            nc.scalar.activation(out=hsb[:], in_=hp[:],
                                 func=mybir.ActivationFunctionType.Relu)
            nc.sync.dma_start(out=hout[w * 128:(w + 1) * 128, :], in_=hsb[:])
            if h1p_local is not None:
                hbf = ep.tile([128, H], mybir.dt.bfloat16, name="hbf",
                              tag="hbf")
                nc.scalar.activation(out=hbf[:], in_=hp[:],
                                     func=mybir.ActivationFunctionType.Relu,
                                     scale=dinvcol_t[:, w:w + 1])
                nc.sync.dma_start(out=h1p_local[w * 128:(w + 1) * 128, :],
                                  in_=hbf[:])


def _build_nc(sched1, tot1, sched2, tot2):
    nc = bacc.Bacc("TRN2", num_swdge_queues=4)

    # inputs (per-core data under the same names)
    x4 = [nc.declare_dram_parameter(f"x{i}", [N // 4, DIN],
                                    mybir.dt.bfloat16, isOutput=False)
          for i in range(4)]
    x_own = nc.declare_dram_parameter("x_own", [NPC_PAD, DIN],
                                      mybir.dt.bfloat16, isOutput=False)
    idx1 = nc.declare_dram_parameter("idx1", [128, tot1 // 16],
                                     mybir.dt.int16, isOutput=False)
    cr1 = nc.declare_dram_parameter("cr1", [128, tot1 // 128],
                                    mybir.dt.bfloat16, isOutput=False)
    idx2 = nc.declare_dram_parameter("idx2", [128, tot2 // 16],
                                     mybir.dt.int16, isOutput=False)
    cr2 = nc.declare_dram_parameter("cr2", [128, tot2 // 128],
                                    mybir.dt.bfloat16, isOutput=False)
    dinvrep = nc.declare_dram_parameter("dinvrep", [128, NPC_PAD],
                                        mybir.dt.float32, isOutput=False)
    dinvcol = nc.declare_dram_parameter("dinvcol", [128, NWIN],
                                        mybir.dt.float32, isOutput=False)
    Wb1 = nc.declare_dram_parameter("Wb1", [DIN, H], mybir.dt.bfloat16,
                                    isOutput=False)
    Wb2 = nc.declare_dram_parameter("Wb2", [H, H], mybir.dt.bfloat16,
                                    isOutput=False)
    bb = nc.declare_dram_parameter("bb", [2, H], mybir.dt.bfloat16,
                                   isOutput=False)
    iota_in = nc.declare_dram_parameter("iota", [128, 8 * WGD],
                                        mybir.dt.bfloat16, isOutput=False)
    h1out = nc.declare_dram_parameter("h1out", [NPC_PAD, H],
                                      mybir.dt.bfloat16, isOutput=True)
    h2out = nc.declare_dram_parameter("h2out", [NPC_PAD, H],
                                      mybir.dt.bfloat16, isOutput=True)

    with tile.TileContext(nc) as tc:
        with tc.tile_pool(name="sb", bufs=3) as sb, \
             tc.tile_pool(name="ip", bufs=6) as ip, \
             tc.tile_pool(name="msk", bufs=6) as msk, \
             tc.tile_pool(name="psum", bufs=2, space="PSUM") as psum, \
             tc.tile_pool(name="ep", bufs=3) as ep, \
             tc.tile_pool(name="const", bufs=1) as cst, \
             tc.tile_pool(name="dram", bufs=1, space="DRAM") as dram:

            h1p_local = dram.tile([NPC_PAD, H], mybir.dt.bfloat16,
                                  name="h1p_local")
            h1p_full = dram.tile([NCORES * NPC_PAD, H], mybir.dt.bfloat16,
                                 name="h1p_full", addr_space="Shared")

            iota_t = cst.tile([128, 8 * WGD], mybir.dt.bfloat16,
                              name="iota_t")
            nc.sync.dma_start(out=iota_t[:], in_=iota_in[:])
            ones_t = cst.tile([1, 128], mybir.dt.bfloat16, name="ones_t")
            nc.vector.memset(ones_t[:], 1.0)
            W1_t = cst.tile([DIN, H], mybir.dt.bfloat16, name="W1_t")
            nc.sync.dma_start(out=W1_t[:], in_=Wb1[:])
            W2_t = cst.tile([H, H], mybir.dt.bfloat16, name="W2_t")
            nc.sync.dma_start(out=W2_t[:], in_=Wb2[:])
            b1_t = cst.tile([1, H], mybir.dt.bfloat16, name="b1_t")
            nc.sync.dma_start(out=b1_t[:], in_=bb[0:1, :])
            b2_t = cst.tile([1, H], mybir.dt.bfloat16, name="b2_t")
            nc.sync.dma_start(out=b2_t[:], in_=bb[1:2, :])
            dinvrep_t = cst.tile([128, NPC_PAD], mybir.dt.float32,
                                 name="dinvrep_t")
            nc.sync.dma_start(out=dinvrep_t[:], in_=dinvrep[:])
            dinvcol_t = cst.tile([128, NWIN], mybir.dt.float32,
                                 name="dinvcol_t")
            nc.sync.dma_start(out=dinvcol_t[:], in_=dinvcol[:])

            pools = (sb, ip, msk, psum, ep)
            qctr = [0]
            _emit_layer(nc, tc, pools, sched1,
                        [t[:] for t in x4] + [x_own[:]], idx1, cr1,
                        iota_t, ones_t, W1_t, b1_t, dinvrep_t, dinvcol_t,
                        h1out, h1p_local, qctr)

            nc.gpsimd.collective_compute(
                "AllGather", mybir.AluOpType.bypass,
                replica_groups=[list(range(NCORES))],
                ins=[h1p_local.opt()], outs=[h1p_full.opt()])

            tbl2_rows = NCORES * NPC_PAD // 4
            tables2 = [h1p_full[i * tbl2_rows:(i + 1) * tbl2_rows]
                       for i in range(4)] + [h1p_local[:]]
            _emit_layer(nc, tc, pools, sched2,
                        tables2, idx2, cr2,
                        iota_t, ones_t, W2_t, b2_t, dinvrep_t, dinvcol_t,
                        h2out, None, qctr)

    nc.finalize()
    return nc


def kernel(x, edge_index, W1, b1, W2, b2):
    global LAST_EXEC_NS, LAST_RESULTS
    x = np.asarray(x, np.float32)
    edge_index = np.asarray(edge_index)
    W1 = np.asarray(W1, np.float32)
    b1 = np.asarray(b1, np.float32)
    W2 = np.asarray(W2, np.float32)
    b2 = np.asarray(b2, np.float32)
    n = x.shape[0]

    row = edge_index[0].astype(np.int64)
    col = edge_index[1].astype(np.int64)
    loop = np.arange(n, dtype=np.int64)
    row_sl = np.concatenate([row, loop])
    col_sl = np.concatenate([col, loop])
    deg = np.bincount(col_sl, minlength=n).astype(np.float64)
    dinv = np.where(deg > 0, 1.0 / np.sqrt(deg), 0.0).astype(np.float32)

    is_self = np.concatenate([np.zeros(len(row), bool), np.ones(n, bool)])
    core = col_sl // NPC
    ecol_l, esrc1_l, esrc2_l = [], [], []
    for c in range(NCORES):
        m = core == c
        sl = is_self[m]
        ecol_l.append(col_sl[m] - c * NPC)
        r = row_sl[m]
        esrc1_l.append(np.where(sl, -1, r))
        # layer-2 source rows live in the allgathered padded layout
        esrc2_l.append(np.where(sl, -1, (r // NPC) * NPC_PAD + (r % NPC)))

    sched1, tot1, idx1, cr1 = _plan_layer(ecol_l, esrc1_l, N // 4)
    sched2, tot2, idx2, cr2 = _plan_layer(ecol_l, esrc2_l,
                                          NCORES * NPC_PAD // 4)

    nc = _build_nc(sched1, tot1, sched2, tot2)

    xb = (x * dinv[:, None].astype(np.float32)).astype(BF16)
    Wb1v = W1.astype(BF16)
    Wb2v = W2.astype(BF16)
    bbv = np.stack([b1, b2]).astype(BF16)
    iota = np.tile(np.arange(WGD, dtype=np.float32).astype(BF16), (128, 8))

    in_maps = []
    for c in range(NCORES):
        m = {f"x{i}": xb[i * (N // 4):(i + 1) * (N // 4)] for i in range(4)}
        xo = np.zeros((NPC_PAD, DIN), BF16)
        xo[:NPC] = xb[c * NPC:(c + 1) * NPC]
        dloc = np.zeros(NPC_PAD, np.float32)
        dloc[:NPC] = dinv[c * NPC:(c + 1) * NPC]
        m.update(x_own=xo, idx1=idx1[c], cr1=cr1[c],
                 idx2=idx2[c], cr2=cr2[c],
                 dinvrep=np.tile(dloc, (128, 1)),
                 dinvcol=np.ascontiguousarray(
                     dloc.reshape(NWIN, 128).T),
                 Wb1=Wb1v, Wb2=Wb2v, bb=bbv, iota=iota)
        in_maps.append(m)

    res = run_bass_kernel_spmd(
        nc, in_maps, core_ids=list(range(NCORES)),
        trace=bool(int(os.environ.get("BASS_TRACE_KERNEL", "0"))))
    LAST_EXEC_NS = res.exec_time_ns
    LAST_RESULTS = res

    h1 = np.concatenate([res.results[c]["h1out"][:NPC].astype(np.float32)
                         for c in range(NCORES)])
    h2 = np.concatenate([res.results[c]["h2out"][:NPC].astype(np.float32)
                         for c in range(NCORES)])
    return np.concatenate([h1, h2], axis=1).astype(np.float32)
